# revision 1
# baseline (speedup 1.0000x reference)
"""AQT quantized einsum 'bsd,df->bsf' on 8 TRN2 NeuronCores.

Math (per reference):
  q_lhs = clip(round(lhs / (rowmax(|lhs|)/127)))   per (b,s) row over d
  q_rhs = clip(round(rhs / (colmax(|rhs|)/127)))   per f column over d
  out   = (q_lhs @ q_rhs) * s_lhs * s_rhs

Sharding: rhs columns (f=16384) split across 8 cores (2048 each); lhs
replicated. Each core computes [8192,4096] @ [4096,2048] in bf16 (the
quantized values are integers <=127, exact in bf16; accumulation stays
far below 2^24 so fp32 PSUM accumulation is exact).

Per-core dataflow:
  - lhs streamed in natural [m,d] layout; per-row absmax (DVE reduce),
    round via the +/-2^23*1.5 magic add, bf16 result staged to DRAM,
    read back transposed through the XBAR DMA (d on partitions).
  - rhs shard streamed twice: pass 1 accumulates elementwise abs-max over
    the d-tiles, PE transposes give per-column absmax; pass 2 quantizes
    with the partition-broadcast reciprocal scale. Half of q_rhs stays
    SBUF-resident per matmul pass.
  - matmul: 32-step k accumulation in PSUM, dequant fused into the
    PSUM->SBUF eviction via scalar_tensor_tensor (one DVE op).
"""

import sys

sys.path.insert(0, "/opt/trn_rl_repo")

import numpy as np

import concourse.bass as bass  # noqa: F401
import concourse.mybir as mybir
import concourse.tile as tile
from concourse import bacc
from concourse.bass_utils import run_bass_kernel_spmd
from concourse.masks import make_identity
from concourse.tile import add_dep_helper

P = 128
B, S, D, F = 4, 2048, 4096, 16384
M = B * S                    # 8192 lhs rows
NC = 8                       # cores
FS = F // NC                 # 2048 rhs columns per core
SUPM = 512                   # m rows per xbar super-block
MAGIC = 12582912.0           # 1.5 * 2^23, fp32 round-to-int trick
QMAX = 127.0

f32 = mybir.dt.float32
bf16 = mybir.dt.bfloat16


def build(m=M, d=D, fs=FS, debug=False, xbar_queues=1, host_qlt=False):
    kt = d // P              # k tiles
    mt = m // P              # m tiles
    nsup = m // SUPM         # super blocks
    msub = SUPM // P
    nhalf = fs // 2          # resident q_rhs columns
    nchunk = min(512, nhalf)
    ncc = nhalf // nchunk    # matmul chunks per half
    fsp = fs // P            # column tiles of the shard

    nc = bacc.Bacc(None, target_bir_lowering=False)
    lhs = nc.dram_tensor("lhs", [m, d], f32, kind="ExternalInput")
    qlt = (nc.dram_tensor("qlt", [d, m], bf16, kind="ExternalInput")
           if host_qlt else None)
    rhs = nc.dram_tensor("rhs", [d, fs], f32, kind="ExternalInput")
    out = nc.dram_tensor("out", [m, fs], f32, kind="ExternalOutput")
    if debug:
        dbg_q = nc.dram_tensor("dbg_q", [m, d], f32, kind="ExternalOutput")
        dbg_qr = nc.dram_tensor("dbg_qr", [d, fs], f32, kind="ExternalOutput")
        dbg_deq = nc.dram_tensor("dbg_deq", [P, fs], f32, kind="ExternalOutput")
        dbg_acol = nc.dram_tensor("dbg_acol", [P, mt], f32, kind="ExternalOutput")
        dbg_qt = nc.dram_tensor("dbg_qt", [P, kt, SUPM], f32, kind="ExternalOutput")
        dbg_qt2 = nc.dram_tensor("dbg_qt2", [P, kt, SUPM], f32, kind="ExternalOutput")

    with tile.TileContext(nc) as tc:
        with (
            tc.tile_pool(name="dram", bufs=1, space="DRAM") as dram,
            tc.tile_pool(name="const", bufs=1) as const,
            tc.tile_pool(name="persist", bufs=1) as persist,
            tc.tile_pool(name="tin", bufs=2) as tin,
            tc.tile_pool(name="tmp", bufs=2) as tmp,
            tc.tile_pool(name="qm", bufs=2) as qmp,
            tc.tile_pool(name="qt", bufs=2) as qtp,
            tc.tile_pool(name="outp", bufs=1) as outp,
            tc.tile_pool(name="pst", bufs=1, space="PSUM") as pst,
            tc.tile_pool(name="psmm", bufs=1, space="PSUM") as psmm,
        ):
            # ---- constants / persistent tiles ----
            ident = const.tile([P, P], f32, tag="ident")
            make_identity(nc, ident)

            d_deq = persist.tile([P, fs], f32, tag="d_deq")      # s_rhs/127 bcast
            acol_s = persist.tile([P, mt], f32, tag="acol")      # lhs absmax/127
            q_rhs_h = persist.tile([P, kt, nhalf], bf16, tag="qrh")
            s_t = persist.tile([fsp, P], f32, tag="s_t")         # colmax, f-major
            r_t = persist.tile([fsp, P], f32, tag="r_t")         # 127/colmax
            ssc_t = persist.tile([fsp, P], f32, tag="ssc_t")     # colmax/127

            # DRAM staging
            q_sup = [dram.tile([SUPM, d], bf16, name=f"q_sup{i}") for i in range(nsup)]
            qr1_dram = dram.tile([kt, P, nhalf], bf16, name="qr1_dram")

            qm_w = [None] * mt       # q_sup write DMA instructions
            qr1_w = [None] * kt      # qr1_dram write DMA instructions

            # ---- lhs quantize (super 0 first for priority, rest later) ----
            def lhs_quant_tile(i, throttle=None):
                lt = tin.tile([P, d], f32, tag="tin", name=f"lt{i}")
                ldma = nc.sync.dma_start(lt[:], lhs[i * P:(i + 1) * P, :])
                if throttle is not None:
                    add_dep_helper(ldma.ins, throttle.ins)
                a = tmp.tile([P, 1], f32, tag="a", name=f"a{i}")
                nc.vector.reduce_max(
                    a[:], lt[:], axis=mybir.AxisListType.X, apply_absolute_value=True
                )
                r = tmp.tile([P, 1], f32, tag="r", name=f"r{i}")
                nc.vector.reciprocal(r[:], a[:])
                nc.vector.tensor_scalar(
                    acol_s[:, i:i + 1], a[:], 1.0 / QMAX, None,
                    mybir.AluOpType.mult,
                )
                r127 = tmp.tile([P, 1], f32, tag="r127", name=f"r127_{i}")
                nc.vector.tensor_scalar(
                    r127[:], r[:], QMAX, None, mybir.AluOpType.mult,
                )
                # t = lhs * (127/absmax) + MAGIC  (in-place on ACT, fp32)
                nc.scalar.activation(
                    lt[:], lt[:], mybir.ActivationFunctionType.Copy,
                    bias=MAGIC, scale=r127[:],
                )
                q = qmp.tile([P, d], bf16, tag="qm", name=f"q{i}")
                # q = (t - MAGIC) cast to bf16 (exact: small integers)
                nc.scalar.activation(
                    q[:], lt[:], mybir.ActivationFunctionType.Copy, bias=-MAGIC
                )
                s, j = i // msub, i % msub
                qm_w[i] = nc.gpsimd.dma_start(
                    q_sup[s][j * P:(j + 1) * P, :], q[:]
                )
                if debug:
                    nc.gpsimd.dma_start(dbg_q[i * P:(i + 1) * P, :], q[:])

            for i in range(min(msub, mt)):
                lhs_quant_tile(i)

            # ---- rhs pass 1: elementwise abs-max over k tiles ----
            mx = persist.tile([P, fs], f32, tag="mx")
            nc.gpsimd.memset(mx[:], 0.0)
            for k in range(kt):
                rt = tin.tile([P, fs], f32, tag="tin", name=f"rt{k}")
                nc.scalar.dma_start(rt[:], rhs[k * P:(k + 1) * P, :])
                nc.scalar.activation(
                    rt[:], rt[:], mybir.ActivationFunctionType.Abs
                )
                nc.vector.tensor_tensor(
                    mx[:], rt[:], mx[:], mybir.AluOpType.max
                )

            # ---- per-column absmax via PE transpose ----
            s_cols = persist.tile([P, fsp], f32, tag="s_cols")
            for j in range(fsp):
                pt = pst.tile([P, P], f32, tag="pst", name=f"pt{j}")
                nc.tensor.transpose(pt[:], mx[:, j * P:(j + 1) * P], ident[:])
                nc.vector.reduce_max(
                    s_cols[:, j:j + 1], pt[:], axis=mybir.AxisListType.X
                )
            pt2 = pst.tile([fsp, P], f32, tag="pst2")
            nc.tensor.transpose(pt2[:], s_cols[:], ident[:])
            nc.vector.tensor_copy(s_t[:], pt2[:])
            nc.vector.reciprocal(r_t[:], s_t[:])
            nc.vector.tensor_scalar(
                r_t[:], r_t[:], QMAX, None, mybir.AluOpType.mult
            )
            nc.vector.tensor_scalar(
                ssc_t[:], s_t[:], 1.0 / QMAX, None, mybir.AluOpType.mult
            )

            # ---- broadcast scales across partitions (log doubling) ----
            b_q = persist.tile([P, fs], f32, tag="mx")
            for j in range(fsp):
                nc.gpsimd.dma_start(b_q[0:1, j * P:(j + 1) * P], r_t[j:j + 1, :])
                nc.gpsimd.dma_start(d_deq[0:1, j * P:(j + 1) * P], ssc_t[j:j + 1, :])
            step = 1
            while step < P:
                nc.gpsimd.dma_start(b_q[step:2 * step, :], b_q[0:step, :])
                nc.gpsimd.dma_start(d_deq[step:2 * step, :], d_deq[0:step, :])
                step *= 2
            if debug:
                nc.gpsimd.dma_start(dbg_deq[:, :], d_deq[:])

            # ---- rhs pass 2: quantize ----
            last_rhs_load = None
            for k in range(kt):
                rt = tin.tile([P, fs], f32, tag="tin", name=f"rq{k}")
                last_rhs_load = nc.scalar.dma_start(rt[:], rhs[k * P:(k + 1) * P, :])
                for h in range(2):
                    sl = slice(h * nhalf, (h + 1) * nhalf)
                    t = tmp.tile([P, nhalf], f32, tag="tq", name=f"tq{k}_{h}")
                    nc.vector.tensor_tensor(
                        t[:], rt[:, sl], b_q[:, sl], mybir.AluOpType.mult
                    )
                    if h == 0:
                        qdst = q_rhs_h[:, k, :]
                        nc.vector.tensor_scalar(
                            qdst, t[:], MAGIC, MAGIC,
                            mybir.AluOpType.add, mybir.AluOpType.subtract,
                        )
                        if debug:
                            nc.gpsimd.dma_start(
                                dbg_qr[k * P:(k + 1) * P, sl], q_rhs_h[:, k, :]
                            )
                    else:
                        q1 = tmp.tile([P, nhalf], bf16, tag="tq", name=f"q1_{k}")
                        nc.vector.tensor_scalar(
                            q1[:], t[:], MAGIC, MAGIC,
                            mybir.AluOpType.add, mybir.AluOpType.subtract,
                        )
                        qr1_w[k] = nc.gpsimd.dma_start(qr1_dram[k], q1[:])
                        if debug:
                            nc.gpsimd.dma_start(
                                dbg_qr[k * P:(k + 1) * P, sl], q1[:]
                            )

            # ---- remaining lhs tiles (throttled behind rhs prologue
            # stream so the rhs pipeline gets full HBM bandwidth) ----
            for i in range(min(msub, mt), mt):
                lhs_quant_tile(
                    i, throttle=last_rhs_load if i >= 3 * msub else None
                )
            if debug:
                nc.gpsimd.dma_start(dbg_acol[:, :], acol_s[:])

            # ---- matmul + dequant ----
            # PSUM accumulation tiles and output staging tiles are
            # allocated ONCE and cycled explicitly: pool slot rotation
            # does not reliably serialize reuse across interleaved
            # accumulation groups, while same-tile WAR/RAW hazards are
            # tracked.  Explicit dep edges added as well: the start
            # matmul of a group waits for the dequant that drained the
            # previous group in the same bank.
            NPS = 6
            ps_ring = [
                psmm.tile([P, nchunk], f32, tag=f"psb{x}", name=f"psb{x}")
                for x in range(NPS)
            ]
            ps_last_reader = [None] * NPS
            NOUT = 2
            o_ring = [
                outp.tile([P, nchunk], f32, tag=f"ob{x}", name=f"ob{x}")
                for x in range(NOUT)
            ]
            o_last_writer = [None] * NOUT
            gidx = 0
            oidx = 0
            last_mm = []             # per (h, s) iteration
            for h in range(2):
                if h == 1:
                    ld = nc.sync.dma_start(
                        q_rhs_h[:], qr1_dram.rearrange("k p f -> p k f")
                    )
                    for k in range(kt):
                        add_dep_helper(ld.ins, qr1_w[k].ins)
                for s in range(nsup):
                    t_it = h * nsup + s
                    qt = qtp.tile([P, kt, SUPM], bf16, tag="qt", name=f"qt{h}_{s}")
                    if host_qlt:
                        xbars = []
                        for k in range(kt):
                            x = nc.sync.dma_start(
                                qt[:, k, :],
                                qlt[k * P:(k + 1) * P, s * SUPM:(s + 1) * SUPM],
                            )
                            if t_it >= 2:
                                add_dep_helper(x.ins, last_mm[t_it - 2].ins)
                            xbars.append(x)
                    else:
                        # whole super in one chunked transpose:
                        # qt[p, k, r] = q_sup[s][r, 128k + p]
                        x = nc.sync.dma_start_transpose(qt[:, :, :], q_sup[s][:, :])
                        for j in range(msub):
                            add_dep_helper(x.ins, qm_w[s * msub + j].ins)
                        if t_it >= 2:
                            add_dep_helper(x.ins, last_mm[t_it - 2].ins)
                        xbars = [x] * kt
                    if debug and h == 0 and s == 0:
                        dbi = nc.gpsimd.dma_start(dbg_qt[:, :, :], qt[:])
                        for x in xbars:
                            add_dep_helper(dbi.ins, x.ins)
                    if debug and h == 0 and s == 7:
                        dbi = nc.gpsimd.dma_start(dbg_qt2[:, :, :], qt[:])
                        for x in xbars:
                            add_dep_helper(dbi.ins, x.ins)
                    mm = None
                    for j in range(msub):
                        mt_idx = s * msub + j
                        slots = []
                        for cc in range(ncc):
                            slots.append(gidx % NPS)
                            gidx += 1
                        for cc in range(ncc):
                            ps = ps_ring[slots[cc]]
                            for k in range(kt):
                                mm = nc.tensor.matmul(
                                    ps[:],
                                    qt[:, k, j * P:(j + 1) * P],
                                    q_rhs_h[:, k, cc * nchunk:(cc + 1) * nchunk],
                                    start=(k == 0),
                                    stop=(k == kt - 1),
                                )
                                add_dep_helper(mm.ins, xbars[k].ins)
                                if k == 0 and ps_last_reader[slots[cc]] is not None:
                                    add_dep_helper(
                                        mm.ins, ps_last_reader[slots[cc]].ins
                                    )
                        for cc in range(ncc):
                            col0 = h * nhalf + cc * nchunk
                            osl = oidx % NOUT
                            oidx += 1
                            o = o_ring[osl]
                            dq = nc.scalar.activation(
                                o[:], ps_ring[slots[cc]][:],
                                mybir.ActivationFunctionType.Copy,
                                bias=0.0, scale=acol_s[:, mt_idx:mt_idx + 1],
                            )
                            nc.vector.tensor_tensor(
                                o[:], o[:], d_deq[:, col0:col0 + nchunk],
                                mybir.AluOpType.mult,
                            )
                            ps_last_reader[slots[cc]] = dq
                            if o_last_writer[osl] is not None:
                                add_dep_helper(dq.ins, o_last_writer[osl].ins)
                            ow = nc.gpsimd.dma_start(
                                out[mt_idx * P:(mt_idx + 1) * P, col0:col0 + nchunk],
                                o[:],
                            )
                            o_last_writer[osl] = ow
                    last_mm.append(mm)
    nc.compile()
    return nc


_nc_cache = None


def _get_nc():
    global _nc_cache
    if _nc_cache is None:
        _nc_cache = build()
    return _nc_cache


def make_in_maps(lhs, rhs):
    lhs2 = np.ascontiguousarray(lhs.reshape(M, D).astype(np.float32))
    return [
        {
            "lhs": lhs2,
            "rhs": np.ascontiguousarray(rhs[:, c * FS:(c + 1) * FS].astype(np.float32)),
        }
        for c in range(NC)
    ]


def kernel(lhs, rhs):
    nc = _get_nc()
    in_maps = make_in_maps(lhs, rhs)
    res = run_bass_kernel_spmd(nc, in_maps, core_ids=list(range(NC)))
    outs = [res.results[c]["out"] for c in range(NC)]
    full = np.concatenate(outs, axis=1)  # [M, F]
    return full.reshape(B, S, F).astype(np.float32)



# revision 6
# speedup vs baseline: 1.2912x; 1.2912x over previous
"""AQT quantized einsum 'bsd,df->bsf' on 8 TRN2 NeuronCores.

Math (per reference):
  q_lhs = round(lhs / (rowmax(|lhs|)/127))   per (b,s) row over d
  q_rhs = round(rhs / (colmax(|rhs|)/127))   per f column over d
  out   = (q_lhs @ q_rhs) * s_lhs * s_rhs

Sharding: rhs columns (f=16384) split across 8 cores (2048 each); lhs
replicated. Each core computes [8192,4096] @ [4096,2048] in bf16 (the
quantized values are integers <=127, exact in bf16; fp32 PSUM
accumulation stays well below 2^24 rounding trouble).

v2 dataflow (vs v1): the full q_rhs shard stays SBUF-resident
(128 KB/partition), so q_lhs^T is consumed ONCE; the transpose happens
on-chip via the SBUF->SBUF XBAR DMA (no DRAM round-trip). Per-column
rhs absmax uses gpsimd partition_all_reduce (no PE transposes, no PSUM
use outside the matmul ring). Matmul sweep: per 128-row lhs tile, one
[128,32,128] transposed tile feeds 32x4 matmuls accumulating four
[128,512] PSUM banks; dequant is fused into the PSUM eviction
(ACT per-row scale, DVE per-column scale).
"""

import sys

sys.path.insert(0, "/opt/trn_rl_repo")

import numpy as np

import concourse.bass as bass  # noqa: F401
import concourse.mybir as mybir
import concourse.tile as tile
from concourse import bacc
from concourse import bass_isa
from concourse.bass_utils import run_bass_kernel_spmd
from concourse.tile import add_dep_helper

P = 128
B, S, D, F = 4, 2048, 4096, 16384
M = B * S                    # 8192 lhs rows
NC = 8                       # cores
FS = F // NC                 # 2048 rhs columns per core
MAGIC = 12582912.0           # 1.5 * 2^23, fp32 round-to-int trick
QMAX = 127.0

f32 = mybir.dt.float32
bf16 = mybir.dt.bfloat16


def build(m=M, d=D, fs=FS):
    kt = d // P              # 32 contraction tiles
    mt = m // P              # 64 lhs row tiles
    nstrip = kt // 2         # rhs strips: two k-tiles side by side
    NCHUNK = 512             # matmul moving width / PSUM bank
    ncc = fs // NCHUNK       # 4 column chunks
    NPS = 8                  # PSUM banks (the whole PSUM)

    nc = bacc.Bacc(None, target_bir_lowering=False)
    lhs = nc.dram_tensor("lhs", [m, d], f32, kind="ExternalInput")
    rhs = nc.dram_tensor("rhs", [d, fs], f32, kind="ExternalInput")
    out = nc.dram_tensor("out", [m, fs], f32, kind="ExternalOutput")

    with tile.TileContext(nc) as tc:
        with (
            tc.tile_pool(name="persist", bufs=1) as persist,
            tc.tile_pool(name="big", bufs=2) as big,      # [P,4096] f32 tiles
            tc.tile_pool(name="q8k", bufs=1) as q8k,      # 8 KB scratch/q tiles
            tc.tile_pool(name="qtp", bufs=2) as qtp,
            tc.tile_pool(name="op", bufs=2) as op,
            tc.tile_pool(name="tmp", bufs=2) as tmp,
            tc.tile_pool(name="psmm", bufs=1, space="PSUM") as psmm,
        ):
            # ---- persistent tiles ----
            q_rhs = persist.tile([P, kt, fs], bf16, tag="qrhs")
            d_deq = persist.tile([P, fs], bf16, tag="ddeq")    # colmax/127
            acol = persist.tile([P, mt], f32, tag="acol")      # rowmax/127
            mx = persist.tile([P, fs], f32, tag="mx")          # absmax acc -> b_q

            # =========== lhs tiles 0..1 early (fill the pipeline) ==========
            lhs_dma = [None] * mt

            def lhs_quant_tile(i):
                lt = big.tile([P, d], f32, tag="big", name=f"lt{i}")
                lhs_dma[i] = nc.sync.dma_start(lt[:], lhs[i * P:(i + 1) * P, :])
                a = tmp.tile([P, 1], f32, tag="a", name=f"a{i}")
                nc.vector.reduce_max(
                    a[:], lt[:], axis=mybir.AxisListType.X,
                    apply_absolute_value=True,
                )
                r = tmp.tile([P, 1], f32, tag="r", name=f"r{i}")
                nc.vector.reciprocal(r[:], a[:])
                nc.vector.tensor_scalar(
                    acol[:, i:i + 1], a[:], 1.0 / QMAX, None,
                    mybir.AluOpType.mult,
                )
                r127 = tmp.tile([P, 1], f32, tag="r127", name=f"r127_{i}")
                nc.vector.tensor_scalar(
                    r127[:], r[:], QMAX, None, mybir.AluOpType.mult,
                )
                # t = lhs * (127/absmax) + MAGIC  (ACT, in-place fp32)
                nc.scalar.activation(
                    lt[:], lt[:], mybir.ActivationFunctionType.Copy,
                    bias=MAGIC, scale=r127[:],
                )
                # q = t - MAGIC, cast bf16 (exact small integers)
                q = q8k.tile([P, d], bf16, tag="q8k", name=f"q{i}")
                nc.vector.tensor_scalar(
                    q[:], lt[:], MAGIC, None, mybir.AluOpType.subtract,
                )
                return q

            early_q = [lhs_quant_tile(i) for i in range(2)]

            # ================= rhs pass 1: elementwise absmax ==============
            nc.gpsimd.memset(mx[:], 0.0)
            for s in range(nstrip):
                rt = big.tile([P, 2, fs], f32, tag="big", name=f"rs{s}")
                nc.scalar.dma_start(rt[:, 0, :], rhs[2 * s * P:(2 * s + 1) * P, :])
                nc.scalar.dma_start(rt[:, 1, :], rhs[(2 * s + 1) * P:(2 * s + 2) * P, :])
                rv = rt[:].rearrange("p a b -> p (a b)")
                nc.scalar.activation(rv, rv, mybir.ActivationFunctionType.Abs)
                for h in range(2):
                    nc.vector.tensor_tensor(
                        mx[:], rt[:, h, :], mx[:], mybir.AluOpType.max
                    )

            # ---- per-column absmax across partitions, chunk by chunk ----
            # b_q = 127/colmax overwrites mx in place (safe: each chunk's
            # all_reduce reads mx[:, c] before the writeback).
            for c in range(ncc):
                sl = slice(c * NCHUNK, (c + 1) * NCHUNK)
                cm = op.tile([P, NCHUNK], f32, tag="o", name=f"cm{c}")
                nc.gpsimd.partition_all_reduce(
                    cm[:], mx[:, sl], channels=P,
                    reduce_op=bass_isa.ReduceOp.absmax,
                )
                nc.vector.tensor_scalar(
                    d_deq[:, sl], cm[:], 1.0 / QMAX, None, mybir.AluOpType.mult,
                )
                rec = op.tile([P, NCHUNK], f32, tag="o", name=f"rec{c}")
                nc.vector.reciprocal(rec[:], cm[:])
                nc.vector.tensor_scalar(
                    mx[:, sl], rec[:], QMAX, None, mybir.AluOpType.mult,
                )
            b_q = mx

            # ================= rhs pass 2: quantize ========================
            for s in range(nstrip):
                rt = big.tile([P, 2, fs], f32, tag="big", name=f"rq{s}")
                nc.scalar.dma_start(rt[:, 0, :], rhs[2 * s * P:(2 * s + 1) * P, :])
                nc.scalar.dma_start(rt[:, 1, :], rhs[(2 * s + 1) * P:(2 * s + 2) * P, :])
                for h in range(2):
                    k = 2 * s + h
                    t = q8k.tile([P, fs], f32, tag="q8k", name=f"t{k}")
                    nc.vector.tensor_tensor(
                        t[:], rt[:, h, :], b_q[:], mybir.AluOpType.mult
                    )
                    nc.vector.tensor_scalar(
                        q_rhs[:, k, :], t[:], MAGIC, MAGIC,
                        mybir.AluOpType.add, mybir.AluOpType.subtract,
                    )

            # ================= matmul + fused dequant ======================
            ps_ring = [
                psmm.tile([P, NCHUNK], f32, tag=f"psb{x}", name=f"psb{x}")
                for x in range(NPS)
            ]
            ps_last_reader = [None] * NPS
            o_last_writer = [None] * 2
            last_mm = [None] * mt

            for i in range(mt):
                if i >= 2:
                    q = lhs_quant_tile(i)
                else:
                    q = early_q[i]
                qt = qtp.tile([P, kt, P], bf16, tag="qt", name=f"qt{i}")
                x = nc.sync.dma_start_transpose(qt[:, :, :], q[:])
                if i >= 2 and last_mm[i - 2] is not None:
                    add_dep_helper(x.ins, last_mm[i - 2].ins)
                banks = [(4 * i + cc) % NPS for cc in range(ncc)]
                mm = None
                for k in range(kt):
                    for cc in range(ncc):
                        ps = ps_ring[banks[cc]]
                        mm = nc.tensor.matmul(
                            ps[:],
                            qt[:, k, :],
                            q_rhs[:, k, cc * NCHUNK:(cc + 1) * NCHUNK],
                            start=(k == 0),
                            stop=(k == kt - 1),
                        )
                        add_dep_helper(mm.ins, x.ins)
                        if k == 0 and ps_last_reader[banks[cc]] is not None:
                            add_dep_helper(
                                mm.ins, ps_last_reader[banks[cc]].ins
                            )
                last_mm[i] = mm
                for cc in range(ncc):
                    sl = slice(cc * NCHUNK, (cc + 1) * NCHUNK)
                    o = op.tile([P, NCHUNK], f32, tag="o", name=f"o{i}_{cc}")
                    dq = nc.scalar.activation(
                        o[:], ps_ring[banks[cc]][:],
                        mybir.ActivationFunctionType.Copy,
                        bias=0.0, scale=acol[:, i:i + 1],
                    )
                    ps_last_reader[banks[cc]] = dq
                    osl = (2 * i + cc) % 2
                    if o_last_writer[osl] is not None:
                        add_dep_helper(dq.ins, o_last_writer[osl].ins)
                    nc.vector.tensor_tensor(
                        o[:], o[:], d_deq[:, sl], mybir.AluOpType.mult
                    )
                    ow = nc.gpsimd.dma_start(
                        out[i * P:(i + 1) * P, sl], o[:]
                    )
                    o_last_writer[osl] = ow
    nc.compile()
    return nc


_nc_cache = None


def _get_nc():
    global _nc_cache
    if _nc_cache is None:
        _nc_cache = build()
    return _nc_cache


def make_in_maps(lhs, rhs):
    lhs2 = np.ascontiguousarray(lhs.reshape(M, D).astype(np.float32))
    return [
        {
            "lhs": lhs2,
            "rhs": np.ascontiguousarray(rhs[:, c * FS:(c + 1) * FS].astype(np.float32)),
        }
        for c in range(NC)
    ]


def kernel(lhs, rhs):
    nc = _get_nc()
    in_maps = make_in_maps(lhs, rhs)
    res = run_bass_kernel_spmd(nc, in_maps, core_ids=list(range(NC)))
    outs = [res.results[c]["out"] for c in range(NC)]
    full = np.concatenate(outs, axis=1)  # [M, F]
    return full.reshape(B, S, F).astype(np.float32)
